# revision 24
# baseline (speedup 1.0000x reference)
"""Multi-head attention forward on 8 TRN2 NeuronCores.

Problem: x[2,2048,1024] @ {Wq,Wk,Wv}[1024,1024] (+bias) -> 16 heads of 64,
softmax(QK^T/8)V per head, concat -> @Wo[1024,1024] + bo.

Sharding: tensor-parallel over d_hid. Core c owns 2 heads (128 dims):
  - computes Q^T,K^T,V^T slices [128, 4096] from full x^T
  - attention for its (2 batches x 2 heads)
  - partial out = ctx_slice @ Wo[slice_rows] -> [4096, 1024]
Host sums the 8 partials and adds bo (pure reduction, no collectives).

Device layout notes:
  - x^T [1024, 4096] uploaded pre-transposed (host prep) so the contraction
    dim (d_in) lands on SBUF partitions for all projection matmuls.
  - Wq, bq pre-scaled by 1/8 on host (folds the softmax scale).
  - All matmuls run in float32r (fp32 single-pass mode, ~1e-4 rel err).
  - Scores computed transposed (S^T[k,q]) so softmax normalization comes
    from a ones-column augmented V (row 64 of the ctx psum = row sums).
  - PSUM banks: scores h0/h1 double-buffered (4) + ctx accum h0/h1 (2) +
    proj/transpose/outproj slots p0/p1 (2) = 8.
"""

import os
import numpy as np

B, S, D = 2, 2048, 1024
NCORES = 8
HSLICE = D // NCORES          # 128 = 2 heads x 64
KT_PROJ = D // 128            # 8 contraction tiles for projections
NKT = S // 128                # 16 k-tiles per batch for attention
QH = 1024                     # q chunk (2 PSUM banks)
CH = 512                      # matmul free-dim chunk

_cache = {}


def _build():
    import concourse.bacc as bacc
    import concourse.tile as tile
    from concourse import mybir
    from concourse.tile_rust import add_dep_helper

    f32 = mybir.dt.float32
    f32r = mybir.dt.float32r
    f16 = mybir.dt.float16
    AF = mybir.ActivationFunctionType

    nc = bacc.Bacc("TRN2", target_bir_lowering=False, debug=False,
                   num_devices=NCORES)

    xt_d = nc.dram_tensor("xt", [D, B * S], f16, kind="ExternalInput").ap()
    wq_d = nc.dram_tensor("wq", [D, HSLICE], f16, kind="ExternalInput").ap()
    wk_d = nc.dram_tensor("wk", [D, HSLICE], f16, kind="ExternalInput").ap()
    wv_d = nc.dram_tensor("wv", [D, HSLICE], f16, kind="ExternalInput").ap()
    bq_d = nc.dram_tensor("bq", [HSLICE, 1], f32, kind="ExternalInput").ap()
    bk_d = nc.dram_tensor("bk", [HSLICE, 1], f32, kind="ExternalInput").ap()
    bv_d = nc.dram_tensor("bv", [HSLICE, 1], f32, kind="ExternalInput").ap()
    wo_d = nc.dram_tensor("wo", [HSLICE, D], f32r, kind="ExternalInput").ap()
    idt_d = nc.dram_tensor("idt", [128, 128], f32r, kind="ExternalInput").ap()
    ones_d = nc.dram_tensor("ones", [128, 1], f16, kind="ExternalInput").ap()
    out_d = nc.dram_tensor("out", [B * S, D], f32, kind="ExternalOutput").ap()

    with tile.TileContext(nc) as tc:
        with (
            tc.tile_pool(name="wpool", bufs=1) as wpool,
            tc.tile_pool(name="xt", bufs=1) as xtp,
            tc.tile_pool(name="qk", bufs=2) as qkp,
            tc.tile_pool(name="vtmp", bufs=1) as vtp,
            tc.tile_pool(name="vaug", bufs=2) as vap,
            tc.tile_pool(name="et", bufs=2) as etp,
            tc.tile_pool(name="ctx", bufs=2) as ctxp,
            tc.tile_pool(name="norm", bufs=1) as normp,
            tc.tile_pool(name="ost", bufs=3) as ostp,
            tc.tile_pool(name="psS", bufs=1, space="PSUM") as psS,
            tc.tile_pool(name="psC", bufs=1, space="PSUM") as psC,
        ):
            # ---- constants / weights ----
            wq_t, wk_t, wv_t = [], [], []
            for ki in range(KT_PROJ):
                for lst, src, tag in ((wq_t, wq_d, "wq"), (wk_t, wk_d, "wk"),
                                      (wv_t, wv_d, "wv")):
                    t = wpool.tile([128, HSLICE], f16, tag=f"{tag}{ki}")
                    nc.scalar.dma_start(t[:], src[ki * 128:(ki + 1) * 128, :])
                    lst.append(t)
            wo_t = wpool.tile([128, D], f32r, tag="wo")
            nc.scalar.dma_start(wo_t[:], wo_d[:])
            idt = wpool.tile([128, 128], f32r, tag="idt")
            nc.scalar.dma_start(idt[:], idt_d[:])
            ones_t = wpool.tile([128, 1], f16, tag="ones")
            nc.scalar.dma_start(ones_t[:], ones_d[:])
            bq_t = wpool.tile([128, 1], f32, tag="bq")
            nc.scalar.dma_start(bq_t[:], bq_d[:])
            bk_t = wpool.tile([128, 1], f32, tag="bk")
            nc.scalar.dma_start(bk_t[:], bk_d[:])
            bv_t = wpool.tile([128, 1], f32, tag="bv")
            nc.scalar.dma_start(bv_t[:], bv_d[:])

            for b in range(B):
                s0 = b * S
                # ---- load x^T for this batch, column-sliced so the first
                # projection chunk can start after ~1/4 of the data ----
                xts = []
                for ki in range(KT_PROJ):
                    t = xtp.tile([128, S], f16, tag=f"xt{ki}")
                    xts.append(t)
                for c in range(S // CH):
                    for ki in range(KT_PROJ):
                        nc.sync.dma_start(
                            xts[ki][:, c * CH:(c + 1) * CH],
                            xt_d[ki * 128:(ki + 1) * 128,
                                 s0 + c * CH:s0 + (c + 1) * CH])

                # ---- projections: Q^T (split per head, zero-padded), K^T,
                # V^T [128, 2048]. qth[h] has the other head's 64 rows zeroed
                # so scores can run full-K=128 matmuls (keeps the PE array
                # fully active -> HAM stays un-throttled).
                qt0 = qkp.tile([128, S], f16, tag="qt0")
                qt1 = qkp.tile([128, S], f16, tag="qt1")
                qth = [qt0, qt1]
                nc.vector.memset(qt0[64:128, :], 0.0)
                nc.vector.memset(qt1[0:64, :], 0.0)
                kt = qkp.tile([128, S], f16, tag="kt")
                vt = vtp.tile([128, S], f32r, tag="vt")
                for di, (dst, w_t, b_t) in enumerate(
                        ((None, wq_t, bq_t), (kt, wk_t, bk_t), (vt, wv_t, bv_t))):
                    for c in range(S // CH):
                        ps = psS.tile([128, CH], f32,
                                      tag=f"sc{(di * (S // CH) + c) % 2}")
                        for ki in range(KT_PROJ):
                            nc.tensor.matmul(ps[:], w_t[ki][:],
                                             xts[ki][:, c * CH:(c + 1) * CH],
                                             start=(ki == 0),
                                             stop=(ki == KT_PROJ - 1))
                        if dst is None:
                            nc.vector.tensor_scalar_add(
                                qt0[0:64, c * CH:(c + 1) * CH],
                                ps[0:64, :], b_t[0:64, 0:1])
                            nc.vector.tensor_scalar_add(
                                qt1[64:128, c * CH:(c + 1) * CH],
                                ps[64:128, :], b_t[64:128, 0:1])
                        else:
                            nc.vector.tensor_scalar_add(
                                dst[:, c * CH:(c + 1) * CH], ps[:], b_t[:, 0:1])

                # ---- V^T -> V_aug tiles [128, 130] (ones at cols 64, 129) ----
                vaugs = []
                for ki in range(NKT):
                    va = vap.tile([128, 130], f16, tag=f"va{ki}")
                    ps = psS.tile([128, 128], f32r, tag=f"sc{ki % 2}")
                    nc.tensor.transpose(ps[:], vt[:, ki * 128:(ki + 1) * 128],
                                        idt[:])
                    nc.vector.tensor_copy(va[:, 0:64], ps[:, 0:64])
                    nc.vector.tensor_copy(va[:, 65:129], ps[:, 64:128])
                    nc.vector.tensor_copy(va[:, 64:65], ones_t[:])
                    nc.vector.tensor_copy(va[:, 129:130], ones_t[:])
                    vaugs.append(va)

                # ---- attention: both heads interleaved (keeps PE dense) ----
                ctxT = ctxp.tile([128, S], f32r, tag="ctxT")
                for qh in range(S // QH):
                    q0 = qh * QH
                    ctx_ps0 = psC.tile([65, QH], f32, tag="ctx0")
                    ctx_ps1 = psC.tile([65, QH], f32, tag="ctx1")
                    ctx_ps = [ctx_ps0, ctx_ps1]

                    def ctx_step(ki, ets):
                        for h in range(2):
                            for c in range(QH // CH):
                                nc.tensor.matmul(
                                    ctx_ps[h][:, c * CH:(c + 1) * CH],
                                    vaugs[ki][:, h * 65:h * 65 + 65],
                                    ets[h][:, c * CH:(c + 1) * CH],
                                    start=(ki == 0), stop=(ki == NKT - 1))

                    # software pipeline: score pair [ki] runs back-to-back
                    # (row-group concurrent), ctx pair [ki-1] fills the exp
                    # latency.
                    prev = None
                    for ki in range(NKT):
                        scs, ets = [], []
                        for h in range(2):
                            sc = psS.tile([128, QH], f32, tag=f"sc{h}")
                            for c in range(QH // CH):
                                nc.tensor.matmul(
                                    sc[:, c * CH:(c + 1) * CH],
                                    kt[:, ki * 128:(ki + 1) * 128],
                                    qth[h][:, q0 + c * CH:q0 + (c + 1) * CH])
                            scs.append(sc)
                        for h in range(2):
                            et = etp.tile([128, QH], f16, tag=f"et{h}")
                            nc.scalar.activation(et[:], scs[h][:], AF.Exp)
                            ets.append(et)
                        if prev is not None:
                            ctx_step(prev[0], prev[1])
                        prev = (ki, ets)
                    ctx_step(prev[0], prev[1])
                    # normalize: row 64 of ctx_ps holds the softmax sums
                    for h in range(2):
                        hp = h * 64
                        s64 = normp.tile([128, QH], f32, tag="s64")
                        nc.vector.tensor_copy(s64[64:65, :],
                                              ctx_ps[h][64:65, :])
                        r0 = normp.tile([1, QH], f32, tag="r0")
                        nc.gpsimd.dma_start(r0[:], s64[64:65, :])
                        bcs = normp.tile([64, QH], f32, tag="bcs")
                        nc.gpsimd.partition_broadcast(bcs[:], r0[:])
                        bc = normp.tile([64, QH], f32, tag="bc")
                        scr = normp.tile([64, QH], f32, tag="scr")
                        nc.vector.reciprocal_approx_accurate(
                            bc[:], bcs[:], scratch=scr[:])
                        nc.vector.tensor_mul(
                            out=ctxT[hp:hp + 64, q0:q0 + QH],
                            in0=ctx_ps[h][0:64, :], in1=bc[:])

                # ---- out projection: out[s0+st*128 ...] = ctx @ Wo_slice ----
                for st in range(S // 128):
                    for c in range(D // CH):
                        ps = psS.tile([128, CH], f32,
                                      tag=f"sc{(st * 2 + c) % 2}")
                        nc.tensor.matmul(ps[:],
                                         ctxT[:, st * 128:(st + 1) * 128],
                                         wo_t[:, c * CH:(c + 1) * CH])
                        ot = ostp.tile([128, CH], f32, tag="ost")
                        nc.vector.tensor_copy(ot[:], ps[:])
                        nc.scalar.dma_start(
                            out_d[s0 + st * 128:s0 + (st + 1) * 128,
                                  c * CH:(c + 1) * CH], ot[:])

    nc.compile()
    return nc


def _get_nc():
    if "nc" not in _cache:
        _cache["nc"] = _build()
    return _cache["nc"]


def kernel(x, Wq, bq, Wk, bk, Wv, bv, Wo, bo):
    from concourse.bass_utils import run_bass_kernel_spmd

    nc = _get_nc()

    x = np.ascontiguousarray(np.asarray(x, dtype=np.float32))
    xt = np.ascontiguousarray(x.reshape(B * S, D).T)          # [D, B*S]
    idt = np.eye(128, dtype=np.float32)

    in_maps = []
    for c in range(NCORES):
        sl = slice(c * HSLICE, (c + 1) * HSLICE)
        in_maps.append({
            "xt": xt.astype(np.float16),
            "wq": (np.ascontiguousarray(np.asarray(Wq, np.float32)[:, sl]) / 8.0).astype(np.float16),
            "wk": np.ascontiguousarray(np.asarray(Wk, np.float32)[:, sl]).astype(np.float16),
            "wv": np.ascontiguousarray(np.asarray(Wv, np.float32)[:, sl]).astype(np.float16),
            "bq": (np.asarray(bq, np.float32)[sl] / 8.0).reshape(HSLICE, 1),
            "bk": np.asarray(bk, np.float32)[sl].reshape(HSLICE, 1),
            "bv": np.asarray(bv, np.float32)[sl].reshape(HSLICE, 1),
            "wo": np.ascontiguousarray(np.asarray(Wo, np.float32)[sl, :]),
            "idt": idt,
            "ones": np.ones((128, 1), np.float16),
        })

    res = run_bass_kernel_spmd(nc, in_maps, core_ids=list(range(NCORES)),
                               trace=bool(int(os.environ.get("KTRACE", "0"))))
    _cache["last_result"] = res
    acc = res.results[0]["out"].astype(np.float32)
    for c in range(1, NCORES):
        acc += res.results[c]["out"]
    acc += np.asarray(bo, np.float32)[None, :]
    return acc.reshape(B, S, D)
